# revision 35
# baseline (speedup 1.0000x reference)
"""Ensemble MLP surrogate (16 models, 32->64->64->64->8, relu) on 8 TRN2 cores.

Data-parallel over batch, weights replicated.  Feature-on-partition layout,
batch streamed as the matmul moving operand in fp16.  Per double-tile of 1024
batch elements the schedule is epilogue-bound (PSUM->SBUF bias+ReLU on the
Scalar/Vector engines), so the design centers on keeping those two engines
saturated:

  - L1 runs as 8-MM octets (two model-pairs concurrently, all 16 PE array
    tiles busy) via parity-swapped W1 packing for odd pairs; L2/L3 as 4-MM
    quads; L4 packs all 16 models' preds into the full 128 partitions
    (pairs j and j+4 share a col tile via PSUM accumulation), halves
    pinging through a single dedicated PSUM bank OUTSIDE the main
    rotation, so the L1-3 pipeline keeps a clean 8-alloc-per-phase period
    (this alone was worth ~35us: a long-lived L4 tile inside the rotation
    stalls L1 every dtile and HAM-cold-clocks the PE).
  - PSUM (8 banks): php bufs=3 x [128,2,512] (6) + p4 1 x [128,512] (1) +
    stats 1 x [64,512] (1).  Mean/sumsq for TWO dtiles accumulate into the
    stats bank at disjoint 8-row bands via column-shifted selector
    matmuls, so the mean-copy/mean^2/sqrt chain runs once per two dtiles
    on [64,512] instead of per-dtile on [8,1024].  With stp bufs=1 the
    whole chain must be EMITTED before the next group's first sel_mean
    (later-emitted readers of a recycled bank race the new group's MMs).
  - The first write to each PSUM partition-region needs start=True:
    has_written clearing is (written partitions) x (whole bank), NOT
    bank-global — a region first written with start=False accumulates
    onto stale PSUM garbage.  skip_group_check where the sim's
    partition-coarse zero-region tracker would false-flag.
  - Squares (pred^2, mean^2) run on the otherwise-idle GPSIMD engine; the
    E[p^2]-mean^2 subtraction is a -identity accumulating matmul on the PE.
  - Each duo's two bias+ReLU epilogues go to DIFFERENT engines (Scalar /
    Vector) so the php slot frees after the parallel max, not a serial sum.
  - Input and output DMA on sync (HWDGE).
"""

import numpy as np

N_MODELS = 16
IN_DIM = 32
HID = 64
OUT_DIM = 8
BATCH = 131072
N_CORES = 8
B_CORE = BATCH // N_CORES  # 16384
TILE = 512  # PSUM bank limit on matmul out cols (fp32)
DTILE = 2 * TILE  # batch elements per pipeline step
NPAIR = N_MODELS // 2

# wpackr free-dim layout (fp16 matmul operands, 128 partitions)
OFF_W1 = 0  # [128, 8, 64]  odd pairs parity-swapped (rows b,a,b,a)
OFF_W2 = OFF_W1 + NPAIR * 64  # [128, 8, 64] canonical (a; b)
OFF_W3 = OFF_W2 + NPAIR * 64  # [128, 8, 64] odd pairs input-swapped (b; a)
OFF_W4 = OFF_W3 + NPAIR * 64  # [128, 8, 32] pair cols 16(j//4)+8c+o
OFF_SELV = OFF_W4 + NPAIR * 32  # [128, 4, 32] band-v selector (1/16)
OFF_NEGD = OFF_SELV + 4 * 32  # [128, 32] -I32 (rows 0-31)
WR = OFF_NEGD + 32
# wpackb free-dim layout (fp32 biases)
OFF_B1 = 0  # [128, 8]
OFF_B2 = OFF_B1 + NPAIR  # [128, 8] (parity-swapped odd pairs)
OFF_B3 = OFF_B2 + NPAIR  # [128, 8]
OFF_B4 = OFF_B3 + NPAIR  # [128, 1] all-model packed preds bias
WB = OFF_B4 + 1

USE_GPS_SQ = True  # pred squares on GPSIMD (else Vector)
USE_GPS_M2 = True  # mean^2 on GPSIMD (else Vector)
DEBUG_EPSQ = False  # skip negmm; stdt outputs raw E[p^2] for debugging


# epilogue op costs (ns) for greedy ACT/DVE load balancing (HW-measured).
# Note: the resulting Scalar-over-Vector busy skew (~9us/kernel) is optimal;
# equalizing busy by biasing ACT's cost shifts ops to the slower-per-op DVE
# (+~170ns each of total work) and measures worse.
def _act_cost(fd):
    return (315 + fd) / 1.2


def _dve_cost(fd):
    return (208 + fd) / 0.96


def pack_inputs(x, W1, b1, W2, b2, W3, b3, W4, b4, b_core=B_CORE, n_cores=N_CORES):
    """Host-side packing. Returns (xt_per_core list, wpackr fp16, wpackb f32)."""
    f32 = np.float32
    x = np.ascontiguousarray(x, dtype=f32)
    wpack = np.zeros((128, WR), f32)
    wpackb = np.zeros((128, WB), f32)

    w1v = wpack[:, OFF_W1 : OFF_W1 + NPAIR * 64].reshape(128, NPAIR, 64)
    w2v = wpack[:, OFF_W2 : OFF_W2 + NPAIR * 64].reshape(128, NPAIR, 64)
    w3v = wpack[:, OFF_W3 : OFF_W3 + NPAIR * 64].reshape(128, NPAIR, 64)
    w4v = wpack[:, OFF_W4 : OFF_W4 + NPAIR * 32].reshape(128, NPAIR, 32)
    for j in range(NPAIR):
        a, b = 2 * j, 2 * j + 1
        for k in range(4):
            # L1 row-group k: even pairs rows a,b,a,b; odd pairs b,a,b,a so the
            # swapped-col octet positions still land both outputs canonically
            if j % 2 == 0:
                w1v[32 * k : 32 * k + 32, j, :] = W1[a if k % 2 == 0 else b]
            else:
                w1v[32 * k : 32 * k + 32, j, :] = W1[b if k % 2 == 0 else a]
        w2v[0:HID, j, :] = W2[a]
        w2v[HID:128, j, :] = W2[b]
        if j % 2 == 0:  # h2 canonical input
            w3v[0:HID, j, :] = W3[a]
            w3v[HID:128, j, :] = W3[b]
        else:  # h2 swapped input (slot0 = model b)
            w3v[0:HID, j, :] = W3[b]
            w3v[HID:128, j, :] = W3[a]
        # pair j's preds land at partitions 32*(j%4) + 16*(j//4) + 8c + o:
        # all 16 models x 8 outs tile the full 128 partitions of one p4 tile
        w4v[0:HID, j, 16 * (j // 4) : 16 * (j // 4) + OUT_DIM] = W4[a]
        w4v[HID:128, j, 16 * (j // 4) + OUT_DIM : 16 * (j // 4) + 16] = W4[b]

    selv = wpack[:, OFF_SELV : OFF_SELV + 4 * 32].reshape(128, 4, 32)
    negd = wpack[:, OFF_NEGD : OFF_NEGD + 32]
    b4v = wpackb[:, OFF_B4]
    for j in range(NPAIR):
        for c in range(2):  # model-within-pair
            for o in range(OUT_DIM):
                p = 32 * (j % 4) + 16 * (j // 4) + 8 * c + o
                b4v[p] = b4[2 * j + c, o]
                for v in range(4):  # stats band (t'=v//2, h=v%2)
                    selv[p, v, 8 * v + o] = 1.0 / 16.0  # exact in fp16
    for r in range(32):
        negd[r, r] = -1.0
    for j in range(NPAIR):
        a, b = 2 * j, 2 * j + 1
        wpackb[0:HID, OFF_B1 + j] = b1[a]
        wpackb[HID:128, OFF_B1 + j] = b1[b]
        # h2 output of odd pairs is parity-swapped (slot0 = model b)
        wpackb[0:HID, OFF_B2 + j] = b2[a if j % 2 == 0 else b]
        wpackb[HID:128, OFF_B2 + j] = b2[b if j % 2 == 0 else a]
        wpackb[0:HID, OFF_B3 + j] = b3[a]
        wpackb[HID:128, OFF_B3 + j] = b3[b]

    wpack16 = wpack.astype(np.float16)
    x16 = x.astype(np.float16)
    xt_per_core = []
    for c in range(n_cores):
        shard = x16[c * b_core : (c + 1) * b_core]  # [b_core, 32]
        xt = np.ascontiguousarray(np.tile(shard.T, (4, 1)))  # [128, b_core]
        xt_per_core.append(xt)
    return xt_per_core, wpack16, wpackb


def _emit(tc, ctx, xt, wr, wb, meant, stdt, b_core):
    import concourse.bass as bass  # noqa: F401
    from concourse import mybir

    nc = tc.nc
    f32 = mybir.dt.float32
    f16 = mybir.dt.float16
    AF = mybir.ActivationFunctionType
    ALU = mybir.AluOpType

    n_dt = b_core // DTILE

    consts = ctx.enter_context(tc.tile_pool(name="consts", bufs=1))
    xp = ctx.enter_context(tc.tile_pool(name="xp", bufs=4))
    hp = [
        ctx.enter_context(tc.tile_pool(name=f"h{i}p", bufs=10)) for i in range(3)
    ]
    prp = ctx.enter_context(tc.tile_pool(name="prp", bufs=6))
    sqp = ctx.enter_context(tc.tile_pool(name="sqp", bufs=6))
    msp = ctx.enter_context(tc.tile_pool(name="msp", bufs=4))  # mean/std/m2 sbuf
    # PSUM budget (8 banks): php 3x[128,2,512]=6 (hosts L1-3 pair tiles AND
    # the packed L4 preds tile, 25 allocs/dtile) + statsp 2x[64,512]=2
    # (2-dtile mean/sumsq accumulators, double-buffered).
    # (A 4-deep php was tried: the extra slack lets the PE sprint then idle
    # past the HAM window -> 52% cold-clock; 3-deep backpressure paces it.)
    php = ctx.enter_context(tc.tile_pool(name="php", bufs=3, space="PSUM"))
    p4p = ctx.enter_context(tc.tile_pool(name="p4p", bufs=1, space="PSUM"))
    stp = ctx.enter_context(tc.tile_pool(name="stp", bufs=1, space="PSUM"))

    cw = consts.tile([128, WR], f16)
    nc.sync.dma_start(out=cw, in_=wr)
    cwb = consts.tile([128, WB], f32)
    nc.sync.dma_start(out=cwb, in_=wb)
    # tiny warmup activations: trigger both ACT table-set loads at kernel
    # start (overlapped with the const/input DMAs) instead of stalling the
    # first real epilogue ~2.6us
    warm = consts.tile([128, 2], f32, tag="warm")
    nc.scalar.activation(warm[:, 0:1], cwb[:, 0:1], AF.Relu)
    # sqrt warmup reads the relu output so its input is >= 0 (sim asserts)
    nc.scalar.activation(warm[:, 1:2], warm[:, 0:1], AF.Sqrt)
    w1v = cw[:, OFF_W1 : OFF_W1 + NPAIR * 64].rearrange("p (j f) -> p j f", f=64)
    w2v = cw[:, OFF_W2 : OFF_W2 + NPAIR * 64].rearrange("p (j f) -> p j f", f=64)
    w3v = cw[:, OFF_W3 : OFF_W3 + NPAIR * 64].rearrange("p (j f) -> p j f", f=64)
    w4v = cw[:, OFF_W4 : OFF_W4 + NPAIR * 32].rearrange("p (j f) -> p j f", f=32)
    selv = cw[:, OFF_SELV : OFF_SELV + 4 * 32].rearrange("p (v f) -> p v f", f=32)
    negd = cw[0:32, OFF_NEGD : OFF_NEGD + 32]

    # greedy engine balancer for PSUM->SBUF epilogues
    eng_ns = {"act": 0.0, "dve": 0.0}

    def epilogue(out, in_, bias, relu, force=None):
        fd = out.free_size()
        use_act = (
            force == "act"
            if force
            else eng_ns["act"] + _act_cost(fd) <= eng_ns["dve"] + _dve_cost(fd)
        )
        if use_act:
            eng_ns["act"] += _act_cost(fd)
            nc.scalar.activation(
                out, in_, AF.Relu if relu else AF.Identity, bias=bias, scale=1.0
            )
        else:
            eng_ns["dve"] += _dve_cost(fd)
            if relu:
                nc.vector.tensor_scalar(
                    out, in_, bias, 0.0, op0=ALU.add, op1=ALU.max
                )
            else:
                nc.vector.tensor_scalar(out, in_, bias, None, op0=ALU.add)

    def l1_octet(d, xt_t, ph):
        j0, j1 = 2 * d, 2 * d + 1
        for k in range(4):
            rg = 32 * k
            c0 = 64 * (k % 2)  # even pair col half
            c1 = 64 * ((k + 1) % 2)  # odd pair col half (swapped)
            nc.tensor.matmul(
                out=ph[j0][c0 : c0 + 64, k // 2, :],
                lhsT=w1v[rg : rg + 32, j0, :],
                rhs=xt_t[rg : rg + 32, k // 2, :],
                start=True, stop=True, tile_position=(rg, c0),
            )
            nc.tensor.matmul(
                out=ph[j1][c1 : c1 + 64, k // 2, :],
                lhsT=w1v[rg : rg + 32, j1, :],
                rhs=xt_t[rg : rg + 32, k // 2, :],
                start=True, stop=True, tile_position=(rg, c1),
            )

    def mid_quads(d, wv, hin, ph):
        # N is hard-capped at 512 per matmul: the ISA rejects a 2-bank
        # N=1024 out AP (codegen assert 's3d3_mm_num_elements') — verified
        j0, j1 = 2 * d, 2 * d + 1
        for h in range(2):
            nc.tensor.matmul(
                out=ph[j0][0:64, h, :], lhsT=wv[0:64, j0, :],
                rhs=hin[j0][0:64, h, :], start=True, stop=True,
                tile_position=(0, 0),
            )
            nc.tensor.matmul(
                out=ph[j0][64:128, h, :], lhsT=wv[64:128, j0, :],
                rhs=hin[j0][64:128, h, :], start=True, stop=True,
                tile_position=(64, 64),
            )
            nc.tensor.matmul(
                out=ph[j1][64:128, h, :], lhsT=wv[0:64, j1, :],
                rhs=hin[j1][0:64, h, :], start=True, stop=True,
                tile_position=(0, 64),
            )
            nc.tensor.matmul(
                out=ph[j1][0:64, h, :], lhsT=wv[64:128, j1, :],
                rhs=hin[j1][64:128, h, :], start=True, stop=True,
                tile_position=(64, 0),
            )

    def layer_duo(lnum, t, xt_t, hin, hout, d):
        """One duo (2 pairs) of one layer: an 8-MM PE burst + 2 epilogues."""
        wv = (None, w2v, w3v)[lnum - 1]
        boff = (OFF_B1, OFF_B2, OFF_B3)[lnum - 1]
        j0, j1 = 2 * d, 2 * d + 1
        ph = {}
        for j in (j0, j1):
            ph[j] = php.tile(
                [128, 2, TILE], f32, tag="ph", name=f"ph{lnum}_{t}_{j}"
            )
        if lnum == 1:
            l1_octet(d, xt_t, ph)
        else:
            mid_quads(d, wv, hin, ph)
        for idx, j in enumerate((j0, j1)):
            hout[j] = hp[lnum - 1].tile(
                [128, 2, TILE], f16, tag=f"h{lnum}", name=f"h{lnum}_{t}_{j}"
            )
            # the duo's two drains go to DIFFERENT engines so they run in
            # parallel: the php slot frees after max(act, dve) ~1.27us
            # instead of a possible 2.5us serial drain on one engine
            epilogue(
                hout[j].rearrange("p h n -> p (h n)"),
                ph[j].rearrange("p h n -> p (h n)"),
                cwb[:, boff + j : boff + j + 1],
                relu=True,
                force="act" if idx == 0 else "dve",
            )

    def l4_mms(pend, h):
        """Packed L4 for half h of the previous dtile: all 16 models' preds
        tile the full 128 partitions of a 1-bank p4 tile (halves ping
        through the same bank).  Pair j lands at partitions
        32*(j%4)+16*(j//4)+8c+o; pairs j and j+4 share a col tile via PSUM
        accumulation (their lhsT cols are disjoint, rest zeros)."""
        h3 = pend["h3"]
        p4h = p4p.tile([128, TILE], f32, tag="p4", name=f"p4_{pend['t']}_{h}")
        pend[f"p4{h}"] = p4h
        # skip_group_check: the sim's zero-region tracker drops the
        # partition base, so the four partition-disjoint col groups of this
        # bank alias to one flag; HW has_written is per-partition and fine.
        for j in range(NPAIR):
            q = j % 4
            nc.tensor.matmul(
                out=p4h[32 * q : 32 * q + 32, :],
                lhsT=w4v[:, j, :],
                rhs=h3[j][:, h, :],
                start=(j < 4), stop=(j >= 4),
                tile_position=(0, 32 * q),
                skip_group_check=True,
            )

    def l4_pr(pend, h):
        prh = prp.tile([128, TILE], f16, tag="pr", name=f"pr_{pend['t']}_{h}")
        epilogue(
            prh, pend[f"p4{h}"], cwb[:, OFF_B4 : OFF_B4 + 1],
            relu=False, force="act",
        )
        pend[f"pr{h}"] = prh

    def sq_gps(pend, h):
        prh = pend[f"pr{h}"]
        sqh = sqp.tile([128, TILE], f16, tag="sq", name=f"sq_{pend['t']}_{h}")
        if USE_GPS_SQ:
            nc.gpsimd.tensor_mul(sqh, prh, prh)
        else:
            nc.vector.tensor_mul(sqh, prh, prh)
            eng_ns["dve"] += (208 + TILE) / 0.96
        pend[f"sq{h}"] = sqh

    def sel_mean(pend, grp, h):
        """Mean selector MM into the 2-dtile stats bank.  Band v=2t'+h of
        selv places dtile t', half h at partitions 8v..8v+8 (zeros elsewhere
        in the 32-row block accumulate harmlessly)."""
        tp = pend["t"] % 2
        if tp == 0 and h == 0:
            grp["st"] = stp.tile(
                [64, TILE], f32, tag="st", name=f"st_{pend['t'] // 2}"
            )
        nc.tensor.matmul(
            out=grp["st"][0:32, :], lhsT=selv[:, 2 * tp + h, :],
            rhs=pend[f"pr{h}"],
            start=(tp == 0 and h == 0), stop=False, tile_position=(0, 0),
            skip_group_check=True,
        )

    def sel_sq(pend, grp, h):
        """E[p^2] selector MM at stats rows 32-63; deferred until the
        gpsimd square has landed.  First sumsq write must start=True
        (has_written clearing is (written partitions) x (whole bank), so
        the mean rows' start does NOT cover rows 32-63); the last sel_sq
        closes the bank's coarse group so the mean copy passes the sim's
        open-group read check."""
        tp = pend["t"] % 2
        nc.tensor.matmul(
            out=grp["st"][32:64, :], lhsT=selv[:, 2 * tp + h, :],
            rhs=pend[f"sq{h}"],
            start=(tp == 0 and h == 0), stop=(tp == 1 and h == 1),
            tile_position=(0, 32), skip_group_check=True,
        )

    def stats_copy_m2(grp):
        # reads rows 0-63 (mean + open sumsq) so the tile tracker orders it
        # against the sel_sq PE writes sharing the bank; rows 32-63 unused
        mean_sb = msp.tile([64, TILE], f32, tag="mean")
        epilogue(mean_sb, grp["st"][0:64, :], 0.0, relu=False, force="act")
        grp["mean_sb"] = mean_sb
        m2t = msp.tile([32, TILE], f16, tag="m2")
        if USE_GPS_M2 and not grp.get("last"):
            nc.gpsimd.tensor_mul(m2t, mean_sb[0:32, :], mean_sb[0:32, :])
        else:
            nc.vector.tensor_mul(m2t, mean_sb[0:32, :], mean_sb[0:32, :])
            eng_ns["dve"] += (208 + TILE) / 0.96
        grp["m2t"] = m2t
        for tp in range(2):
            x0 = grp["x0"][tp]
            for h in range(2):
                r = 16 * tp + 8 * h
                nc.sync.dma_start(
                    out=meant[:, x0 + h * TILE : x0 + (h + 1) * TILE],
                    in_=mean_sb[r : r + 8, :],
                )

    def stats_negmm_sqrt(grp):
        st = grp["st"]
        std_sb = msp.tile([32, TILE], f32, tag="std")
        if DEBUG_EPSQ:
            nc.scalar.activation(
                out=std_sb, in_=st[32:64, :], func=AF.Identity, scale=1.0
            )
            eng_ns["act"] += _act_cost(TILE)
            grp["std_sb"] = std_sb
            return
        # st[32:64] -= mean^2  (PE accumulate; the coarse group was already
        # closed by the last sel_sq, so bypass the checker)
        nc.tensor.matmul(
            out=st[32:64, :], lhsT=negd, rhs=grp["m2t"],
            start=False, stop=True, tile_position=(0, 32),
            skip_group_check=True,
        )
        # std = sqrt(16/15 * (E[p^2] - mean^2))
        nc.scalar.activation(
            out=std_sb, in_=st[32:64, :], func=AF.Sqrt, scale=16.0 / 15.0,
        )
        eng_ns["act"] += _act_cost(TILE)
        grp["std_sb"] = std_sb

    def stats_dma(grp):
        for tp in range(2):
            x0 = grp["x0"][tp]
            for h in range(2):
                r = 16 * tp + 8 * h
                nc.sync.dma_start(
                    out=stdt[:, x0 + h * TILE : x0 + (h + 1) * TILE],
                    in_=grp["std_sb"][r : r + 8, :],
                )

    def xt_fetch(t, eng=None):
        x0 = t * DTILE
        tile = xp.tile([128, 2, TILE], f16, tag="xt", name=f"xt_{t}")
        (eng or nc.sync).dma_start(
            out=tile,
            in_=xt[:, x0 : x0 + DTILE].rearrange("p (h n) -> p h n", n=TILE),
        )
        return tile

    # Software pipeline with fine-grained interleaving: dtile t-1's L4,
    # preds, and stats-chain links are woven between dtile t's duo bursts.
    # Each deferred block's dependencies resolved >=2 phases earlier, so it
    # never stalls the PE FIFO head, and it gives the PE dependency-free
    # filler work during epilogue drain waits (keeps HAM warm).
    # Input tiles are prefetched 2 dtiles ahead (xp bufs=3) so L1 never
    # waits on the inbound DMA.
    # One-deep deferral, spread thin: dtile t-1's L4 halves, preds
    # epilogues, squares, and stats selectors are woven through iter t as
    # PE/engine filler (each dep resolved >= 2 phases earlier); the 1-bank
    # p4 tile pings h0/h1 through the same bank outside the php rotation,
    # so the L1-3 rotation stays a clean 8-alloc-per-phase period.  The
    # stats chain for a 2-dtile group runs early in the following
    # iteration — entirely BEFORE the next group's first sel_mean, since
    # with stp bufs=1 a later-emitted reader of the old bank would race
    # the new group's matmuls.
    # startup loads spread across idle engine DGE queues: cw/cwb go on
    # sync, xt0/xt1 on scalar/vector, so the first L1 matmul is gated by
    # ~max(transfer) instead of the serial sum (~3.5us earlier start)
    xt_tiles = {0: xt_fetch(0, nc.scalar), 1: xt_fetch(1, nc.gpsimd)}
    pend = None
    cgrp = None  # group whose copy/m2/negmm/sqrt chain runs this iter
    grp = None
    for t in range(n_dt + 1):
        run = t < n_dt
        if run:
            x0 = t * DTILE
            for pf in (t + 2, t + 3):
                if pf < n_dt and pf not in xt_tiles:
                    xt_tiles[pf] = xt_fetch(pf)
            xt_t = xt_tiles.pop(t)
            h1, h2, h3 = {}, {}, {}
            layer_duo(1, t, xt_t, None, h1, 0)
        if pend is not None:
            l4_mms(pend, 0)
        if run:
            layer_duo(1, t, xt_t, None, h1, 1)
        if pend is not None:
            l4_pr(pend, 0)
        if cgrp is not None:
            stats_copy_m2(cgrp)
        if run:
            layer_duo(1, t, xt_t, None, h1, 2)
        if pend is not None:
            l4_mms(pend, 1)
            sq_gps(pend, 0)
        if run:
            layer_duo(1, t, xt_t, None, h1, 3)
        if pend is not None:
            l4_pr(pend, 1)
        if cgrp is not None:
            stats_negmm_sqrt(cgrp)
        if run:
            layer_duo(2, t, None, h1, h2, 0)
        if pend is not None:
            sq_gps(pend, 1)
            if pend["t"] % 2 == 0:
                grp = {"x0": {}}
            grp["x0"][pend["t"] % 2] = pend["x0"]
            pend["grp"] = grp
            sel_mean(pend, grp, 0)
        if run:
            layer_duo(2, t, None, h1, h2, 1)
        if cgrp is not None:
            stats_dma(cgrp)
        if run:
            layer_duo(2, t, None, h1, h2, 2)
        if pend is not None:
            sel_mean(pend, grp, 1)
        if run:
            layer_duo(2, t, None, h1, h2, 3)
            layer_duo(3, t, None, h2, h3, 0)
        if pend is not None:
            sel_sq(pend, grp, 0)
        if run:
            layer_duo(3, t, None, h2, h3, 1)
            layer_duo(3, t, None, h2, h3, 2)
        if pend is not None:
            sel_sq(pend, grp, 1)
        if run:
            layer_duo(3, t, None, h2, h3, 3)
        cgrp = None
        if pend is not None and pend["t"] % 2 == 1:
            cgrp = pend["grp"]
        if run and t == n_dt - 1:
            # tail compression: the final dtile's L4/preds/stats run inline
            # right after its own L3 instead of one deferred iteration
            # later, so the flush iteration only runs the last stats chain
            sp = {"x0": x0, "t": t, "h3": h3}
            l4_mms(sp, 0)
            l4_pr(sp, 0)
            sq_gps(sp, 0)
            l4_mms(sp, 1)
            l4_pr(sp, 1)
            sq_gps(sp, 1)
            grp["x0"][1] = x0
            sp["grp"] = grp
            sel_mean(sp, grp, 0)
            sel_mean(sp, grp, 1)
            sel_sq(sp, grp, 0)
            sel_sq(sp, grp, 1)
            grp["last"] = True  # chain m2 on the then-idle Vector engine
            cgrp = grp
            pend = None
        elif run:
            pend = {"x0": x0, "t": t, "h3": h3}
        else:
            pend = None


def build(b_core=B_CORE, num_devices=N_CORES):
    from contextlib import ExitStack

    import concourse.bacc as bacc
    import concourse.tile as tile
    from concourse import mybir

    f32 = mybir.dt.float32
    f16 = mybir.dt.float16
    nc = bacc.Bacc(
        "TRN2", target_bir_lowering=False, debug=False, num_devices=num_devices
    )
    xt = nc.dram_tensor("xt", [128, b_core], f16, kind="ExternalInput").ap()
    wr = nc.dram_tensor("wpackr", [128, WR], f16, kind="ExternalInput").ap()
    wb = nc.dram_tensor("wpackb", [128, WB], f32, kind="ExternalInput").ap()
    meant = nc.dram_tensor("meant", [8, b_core], f32, kind="ExternalOutput").ap()
    stdt = nc.dram_tensor("stdt", [8, b_core], f32, kind="ExternalOutput").ap()
    with tile.TileContext(nc) as tc:
        with ExitStack() as ctx:
            _emit(tc, ctx, xt, wr, wb, meant, stdt, b_core)
    nc.compile()
    return nc


_NC_CACHE = {}


def kernel(x, W1, b1, W2, b2, W3, b3, W4, b4):
    from concourse.bass_utils import run_bass_kernel_spmd

    key = ("full", B_CORE)
    if key not in _NC_CACHE:
        _NC_CACHE[key] = build(B_CORE, N_CORES)
    nc = _NC_CACHE[key]

    xt_per_core, wpackr, wpackb = pack_inputs(
        np.asarray(x), np.asarray(W1), np.asarray(b1), np.asarray(W2),
        np.asarray(b2), np.asarray(W3), np.asarray(b3), np.asarray(W4),
        np.asarray(b4),
    )
    in_maps = [
        {"xt": xt_per_core[c], "wpackr": wpackr, "wpackb": wpackb}
        for c in range(N_CORES)
    ]
    # transient device wedges (NRT_EXEC_UNIT_UNRECOVERABLE etc.) recover on
    # retry, per the TRN2 runtime docs; don't let one sink the whole call
    import os
    res = None
    for attempt in range(3):
        try:
            res = run_bass_kernel_spmd(nc, in_maps, list(range(N_CORES))).results
            break
        except Exception:
            if attempt == 2:
                raise
            os.environ.setdefault("NEURON_RT_RESET_CORES", "1")
    mean = np.concatenate([res[c]["meant"] for c in range(N_CORES)], axis=1).T
    std = np.concatenate([res[c]["stdt"] for c in range(N_CORES)], axis=1).T
    return np.ascontiguousarray(mean), np.ascontiguousarray(std)



# revision 36
# speedup vs baseline: 1.0070x; 1.0070x over previous
"""Ensemble MLP surrogate (16 models, 32->64->64->64->8, relu) on 8 TRN2 cores.

Data-parallel over batch, weights replicated.  Feature-on-partition layout,
batch streamed as the matmul moving operand in fp16.  Per double-tile of 1024
batch elements the schedule is epilogue-bound (PSUM->SBUF bias+ReLU on the
Scalar/Vector engines), so the design centers on keeping those two engines
saturated:

  - L1 runs as 8-MM octets (two model-pairs concurrently, all 16 PE array
    tiles busy) via parity-swapped W1 packing for odd pairs; L2/L3 as 4-MM
    quads; L4 packs all 16 models' preds into the full 128 partitions
    (pairs j and j+4 share a col tile via PSUM accumulation), halves
    pinging through a single dedicated PSUM bank OUTSIDE the main
    rotation, so the L1-3 pipeline keeps a clean 8-alloc-per-phase period
    (this alone was worth ~35us: a long-lived L4 tile inside the rotation
    stalls L1 every dtile and HAM-cold-clocks the PE).
  - PSUM (8 banks): php bufs=3 x [128,2,512] (6) + p4 1 x [128,512] (1) +
    stats 1 x [64,512] (1).  Mean/sumsq for TWO dtiles accumulate into the
    stats bank at disjoint 8-row bands via column-shifted selector
    matmuls, so the mean-copy/mean^2/sqrt chain runs once per two dtiles
    on [64,512] instead of per-dtile on [8,1024].  With stp bufs=1 the
    whole chain must be EMITTED before the next group's first sel_mean
    (later-emitted readers of a recycled bank race the new group's MMs).
  - The first write to each PSUM partition-region needs start=True:
    has_written clearing is (written partitions) x (whole bank), NOT
    bank-global — a region first written with start=False accumulates
    onto stale PSUM garbage.  skip_group_check where the sim's
    partition-coarse zero-region tracker would false-flag.
  - Squares (pred^2, mean^2) run on the otherwise-idle GPSIMD engine; the
    E[p^2]-mean^2 subtraction is a -identity accumulating matmul on the PE.
  - Each duo's two bias+ReLU epilogues go to DIFFERENT engines (Scalar /
    Vector) so the php slot frees after the parallel max, not a serial sum.
  - Input and output DMA on sync (HWDGE).
"""

import numpy as np

N_MODELS = 16
IN_DIM = 32
HID = 64
OUT_DIM = 8
BATCH = 131072
N_CORES = 8
B_CORE = BATCH // N_CORES  # 16384
TILE = 512  # PSUM bank limit on matmul out cols (fp32)
DTILE = 2 * TILE  # batch elements per pipeline step
NPAIR = N_MODELS // 2

# wpackr free-dim layout (fp16 matmul operands, 128 partitions)
OFF_W1 = 0  # [128, 8, 64]  odd pairs parity-swapped (rows b,a,b,a)
OFF_W2 = OFF_W1 + NPAIR * 64  # [128, 8, 64] canonical (a; b)
OFF_W3 = OFF_W2 + NPAIR * 64  # [128, 8, 64] odd pairs input-swapped (b; a)
OFF_W4 = OFF_W3 + NPAIR * 64  # [128, 8, 32] pair cols 16(j//4)+8c+o
OFF_SELV = OFF_W4 + NPAIR * 32  # [128, 4, 32] band-v selector (1/16)
OFF_NEGD = OFF_SELV + 4 * 32  # [128, 32] -I32 (rows 0-31)
WR = OFF_NEGD + 32
# wpackb free-dim layout (fp32 biases)
OFF_B1 = 0  # [128, 8]
OFF_B2 = OFF_B1 + NPAIR  # [128, 8] (parity-swapped odd pairs)
OFF_B3 = OFF_B2 + NPAIR  # [128, 8]
OFF_B4 = OFF_B3 + NPAIR  # [128, 1] all-model packed preds bias
WB = OFF_B4 + 1

USE_GPS_SQ = True  # pred squares on GPSIMD (else Vector)
USE_GPS_M2 = True  # mean^2 on GPSIMD (else Vector)
DEBUG_EPSQ = False  # skip negmm; stdt outputs raw E[p^2] for debugging


# epilogue op costs (ns) for greedy ACT/DVE load balancing (HW-measured).
# Note: the resulting Scalar-over-Vector busy skew (~9us/kernel) is optimal;
# equalizing busy by biasing ACT's cost shifts ops to the slower-per-op DVE
# (+~170ns each of total work) and measures worse.
def _act_cost(fd):
    return (315 + fd) / 1.2


def _dve_cost(fd):
    return (208 + fd) / 0.96


def pack_inputs(x, W1, b1, W2, b2, W3, b3, W4, b4, b_core=B_CORE, n_cores=N_CORES):
    """Host-side packing. Returns (xt_per_core list, wpackr fp16, wpackb f32)."""
    f32 = np.float32
    x = np.ascontiguousarray(x, dtype=f32)
    wpack = np.zeros((128, WR), f32)
    wpackb = np.zeros((128, WB), f32)

    w1v = wpack[:, OFF_W1 : OFF_W1 + NPAIR * 64].reshape(128, NPAIR, 64)
    w2v = wpack[:, OFF_W2 : OFF_W2 + NPAIR * 64].reshape(128, NPAIR, 64)
    w3v = wpack[:, OFF_W3 : OFF_W3 + NPAIR * 64].reshape(128, NPAIR, 64)
    w4v = wpack[:, OFF_W4 : OFF_W4 + NPAIR * 32].reshape(128, NPAIR, 32)
    for j in range(NPAIR):
        a, b = 2 * j, 2 * j + 1
        for k in range(4):
            # L1 row-group k: even pairs rows a,b,a,b; odd pairs b,a,b,a so the
            # swapped-col octet positions still land both outputs canonically
            if j % 2 == 0:
                w1v[32 * k : 32 * k + 32, j, :] = W1[a if k % 2 == 0 else b]
            else:
                w1v[32 * k : 32 * k + 32, j, :] = W1[b if k % 2 == 0 else a]
        w2v[0:HID, j, :] = W2[a]
        w2v[HID:128, j, :] = W2[b]
        if j % 2 == 0:  # h2 canonical input
            w3v[0:HID, j, :] = W3[a]
            w3v[HID:128, j, :] = W3[b]
        else:  # h2 swapped input (slot0 = model b)
            w3v[0:HID, j, :] = W3[b]
            w3v[HID:128, j, :] = W3[a]
        # pair j's preds land at partitions 32*(j%4) + 16*(j//4) + 8c + o:
        # all 16 models x 8 outs tile the full 128 partitions of one p4 tile
        w4v[0:HID, j, 16 * (j // 4) : 16 * (j // 4) + OUT_DIM] = W4[a]
        w4v[HID:128, j, 16 * (j // 4) + OUT_DIM : 16 * (j // 4) + 16] = W4[b]

    selv = wpack[:, OFF_SELV : OFF_SELV + 4 * 32].reshape(128, 4, 32)
    negd = wpack[:, OFF_NEGD : OFF_NEGD + 32]
    b4v = wpackb[:, OFF_B4]
    for j in range(NPAIR):
        for c in range(2):  # model-within-pair
            for o in range(OUT_DIM):
                p = 32 * (j % 4) + 16 * (j // 4) + 8 * c + o
                b4v[p] = b4[2 * j + c, o]
                for v in range(4):  # stats band (t'=v//2, h=v%2)
                    selv[p, v, 8 * v + o] = 1.0 / 16.0  # exact in fp16
    for r in range(32):
        negd[r, r] = -1.0
    for j in range(NPAIR):
        a, b = 2 * j, 2 * j + 1
        wpackb[0:HID, OFF_B1 + j] = b1[a]
        wpackb[HID:128, OFF_B1 + j] = b1[b]
        # h2 output of odd pairs is parity-swapped (slot0 = model b)
        wpackb[0:HID, OFF_B2 + j] = b2[a if j % 2 == 0 else b]
        wpackb[HID:128, OFF_B2 + j] = b2[b if j % 2 == 0 else a]
        wpackb[0:HID, OFF_B3 + j] = b3[a]
        wpackb[HID:128, OFF_B3 + j] = b3[b]

    wpack16 = wpack.astype(np.float16)
    x16 = x.astype(np.float16)
    xt_per_core = []
    for c in range(n_cores):
        shard = x16[c * b_core : (c + 1) * b_core]  # [b_core, 32]
        xt = np.ascontiguousarray(np.tile(shard.T, (4, 1)))  # [128, b_core]
        xt_per_core.append(xt)
    return xt_per_core, wpack16, wpackb


def _emit(tc, ctx, xt, wr, wb, meant, stdt, b_core):
    import concourse.bass as bass  # noqa: F401
    from concourse import mybir

    nc = tc.nc
    f32 = mybir.dt.float32
    f16 = mybir.dt.float16
    AF = mybir.ActivationFunctionType
    ALU = mybir.AluOpType

    n_dt = b_core // DTILE

    consts = ctx.enter_context(tc.tile_pool(name="consts", bufs=1))
    xp = ctx.enter_context(tc.tile_pool(name="xp", bufs=4))
    hp = [
        ctx.enter_context(tc.tile_pool(name=f"h{i}p", bufs=10)) for i in range(3)
    ]
    prp = ctx.enter_context(tc.tile_pool(name="prp", bufs=6))
    sqp = ctx.enter_context(tc.tile_pool(name="sqp", bufs=6))
    msp = ctx.enter_context(tc.tile_pool(name="msp", bufs=4))  # mean/std/m2 sbuf
    # PSUM budget (8 banks): php 3x[128,2,512]=6 (hosts L1-3 pair tiles AND
    # the packed L4 preds tile, 25 allocs/dtile) + statsp 2x[64,512]=2
    # (2-dtile mean/sumsq accumulators, double-buffered).
    # (A 4-deep php was tried: the extra slack lets the PE sprint then idle
    # past the HAM window -> 52% cold-clock; 3-deep backpressure paces it.)
    php = ctx.enter_context(tc.tile_pool(name="php", bufs=3, space="PSUM"))
    p4p = ctx.enter_context(tc.tile_pool(name="p4p", bufs=1, space="PSUM"))
    stp = ctx.enter_context(tc.tile_pool(name="stp", bufs=1, space="PSUM"))

    cw = consts.tile([128, WR], f16)
    nc.sync.dma_start(out=cw, in_=wr)
    cwb = consts.tile([128, WB], f32)
    nc.sync.dma_start(out=cwb, in_=wb)
    # tiny warmup activations: trigger both ACT table-set loads at kernel
    # start (overlapped with the const/input DMAs) instead of stalling the
    # first real epilogue ~2.6us
    warm = consts.tile([128, 2], f32, tag="warm")
    nc.scalar.activation(warm[:, 0:1], cwb[:, 0:1], AF.Relu)
    # sqrt warmup reads the relu output so its input is >= 0 (sim asserts)
    nc.scalar.activation(warm[:, 1:2], warm[:, 0:1], AF.Sqrt)
    w1v = cw[:, OFF_W1 : OFF_W1 + NPAIR * 64].rearrange("p (j f) -> p j f", f=64)
    w2v = cw[:, OFF_W2 : OFF_W2 + NPAIR * 64].rearrange("p (j f) -> p j f", f=64)
    w3v = cw[:, OFF_W3 : OFF_W3 + NPAIR * 64].rearrange("p (j f) -> p j f", f=64)
    w4v = cw[:, OFF_W4 : OFF_W4 + NPAIR * 32].rearrange("p (j f) -> p j f", f=32)
    selv = cw[:, OFF_SELV : OFF_SELV + 4 * 32].rearrange("p (v f) -> p v f", f=32)
    negd = cw[0:32, OFF_NEGD : OFF_NEGD + 32]

    # greedy engine balancer for PSUM->SBUF epilogues
    eng_ns = {"act": 0.0, "dve": 0.0}

    def epilogue(out, in_, bias, relu, force=None):
        fd = out.free_size()
        use_act = (
            force == "act"
            if force
            else eng_ns["act"] + _act_cost(fd) <= eng_ns["dve"] + _dve_cost(fd)
        )
        if use_act:
            eng_ns["act"] += _act_cost(fd)
            nc.scalar.activation(
                out, in_, AF.Relu if relu else AF.Identity, bias=bias, scale=1.0
            )
        else:
            eng_ns["dve"] += _dve_cost(fd)
            if relu:
                nc.vector.tensor_scalar(
                    out, in_, bias, 0.0, op0=ALU.add, op1=ALU.max
                )
            else:
                nc.vector.tensor_scalar(out, in_, bias, None, op0=ALU.add)

    def l1_octet(d, xt_t, ph):
        j0, j1 = 2 * d, 2 * d + 1
        for k in range(4):
            rg = 32 * k
            c0 = 64 * (k % 2)  # even pair col half
            c1 = 64 * ((k + 1) % 2)  # odd pair col half (swapped)
            nc.tensor.matmul(
                out=ph[j0][c0 : c0 + 64, k // 2, :],
                lhsT=w1v[rg : rg + 32, j0, :],
                rhs=xt_t[rg : rg + 32, k // 2, :],
                start=True, stop=True, tile_position=(rg, c0),
            )
            nc.tensor.matmul(
                out=ph[j1][c1 : c1 + 64, k // 2, :],
                lhsT=w1v[rg : rg + 32, j1, :],
                rhs=xt_t[rg : rg + 32, k // 2, :],
                start=True, stop=True, tile_position=(rg, c1),
            )

    def mid_quads(d, wv, hin, ph):
        # N is hard-capped at 512 per matmul: the ISA rejects a 2-bank
        # N=1024 out AP (codegen assert 's3d3_mm_num_elements') — verified
        j0, j1 = 2 * d, 2 * d + 1
        for h in range(2):
            nc.tensor.matmul(
                out=ph[j0][0:64, h, :], lhsT=wv[0:64, j0, :],
                rhs=hin[j0][0:64, h, :], start=True, stop=True,
                tile_position=(0, 0),
            )
            nc.tensor.matmul(
                out=ph[j0][64:128, h, :], lhsT=wv[64:128, j0, :],
                rhs=hin[j0][64:128, h, :], start=True, stop=True,
                tile_position=(64, 64),
            )
            nc.tensor.matmul(
                out=ph[j1][64:128, h, :], lhsT=wv[0:64, j1, :],
                rhs=hin[j1][0:64, h, :], start=True, stop=True,
                tile_position=(0, 64),
            )
            nc.tensor.matmul(
                out=ph[j1][0:64, h, :], lhsT=wv[64:128, j1, :],
                rhs=hin[j1][64:128, h, :], start=True, stop=True,
                tile_position=(64, 0),
            )

    def layer_duo(lnum, t, xt_t, hin, hout, d):
        """One duo (2 pairs) of one layer: an 8-MM PE burst + 2 epilogues."""
        wv = (None, w2v, w3v)[lnum - 1]
        boff = (OFF_B1, OFF_B2, OFF_B3)[lnum - 1]
        j0, j1 = 2 * d, 2 * d + 1
        ph = {}
        for j in (j0, j1):
            ph[j] = php.tile(
                [128, 2, TILE], f32, tag="ph", name=f"ph{lnum}_{t}_{j}"
            )
        if lnum == 1:
            l1_octet(d, xt_t, ph)
        else:
            mid_quads(d, wv, hin, ph)
        for idx, j in enumerate((j0, j1)):
            hout[j] = hp[lnum - 1].tile(
                [128, 2, TILE], f16, tag=f"h{lnum}", name=f"h{lnum}_{t}_{j}"
            )
            # the duo's two drains go to DIFFERENT engines so they run in
            # parallel: the php slot frees after max(act, dve) ~1.27us
            # instead of a possible 2.5us serial drain on one engine
            epilogue(
                hout[j].rearrange("p h n -> p (h n)"),
                ph[j].rearrange("p h n -> p (h n)"),
                cwb[:, boff + j : boff + j + 1],
                relu=True,
                force="act" if idx == 0 else "dve",
            )

    def l4_mms(pend, h):
        """Packed L4 for half h of the previous dtile: all 16 models' preds
        tile the full 128 partitions of a 1-bank p4 tile (halves ping
        through the same bank).  Pair j lands at partitions
        32*(j%4)+16*(j//4)+8c+o; pairs j and j+4 share a col tile via PSUM
        accumulation (their lhsT cols are disjoint, rest zeros)."""
        h3 = pend["h3"]
        p4h = p4p.tile([128, TILE], f32, tag="p4", name=f"p4_{pend['t']}_{h}")
        pend[f"p4{h}"] = p4h
        # skip_group_check: the sim's zero-region tracker drops the
        # partition base, so the four partition-disjoint col groups of this
        # bank alias to one flag; HW has_written is per-partition and fine.
        for j in range(NPAIR):
            q = j % 4
            nc.tensor.matmul(
                out=p4h[32 * q : 32 * q + 32, :],
                lhsT=w4v[:, j, :],
                rhs=h3[j][:, h, :],
                start=(j < 4), stop=(j >= 4),
                tile_position=(0, 32 * q),
                skip_group_check=True,
            )

    def l4_pr(pend, h):
        prh = prp.tile([128, TILE], f16, tag="pr", name=f"pr_{pend['t']}_{h}")
        epilogue(
            prh, pend[f"p4{h}"], cwb[:, OFF_B4 : OFF_B4 + 1],
            relu=False, force="act",
        )
        pend[f"pr{h}"] = prh

    def sq_gps(pend, h):
        prh = pend[f"pr{h}"]
        sqh = sqp.tile([128, TILE], f16, tag="sq", name=f"sq_{pend['t']}_{h}")
        if USE_GPS_SQ and not pend.get("last"):
            nc.gpsimd.tensor_mul(sqh, prh, prh)
        else:
            nc.vector.tensor_mul(sqh, prh, prh)
            eng_ns["dve"] += (208 + TILE) / 0.96
        pend[f"sq{h}"] = sqh

    def sel_mean(pend, grp, h):
        """Mean selector MM into the 2-dtile stats bank.  Band v=2t'+h of
        selv places dtile t', half h at partitions 8v..8v+8 (zeros elsewhere
        in the 32-row block accumulate harmlessly)."""
        tp = pend["t"] % 2
        if tp == 0 and h == 0:
            grp["st"] = stp.tile(
                [64, TILE], f32, tag="st", name=f"st_{pend['t'] // 2}"
            )
        nc.tensor.matmul(
            out=grp["st"][0:32, :], lhsT=selv[:, 2 * tp + h, :],
            rhs=pend[f"pr{h}"],
            start=(tp == 0 and h == 0), stop=False, tile_position=(0, 0),
            skip_group_check=True,
        )

    def sel_sq(pend, grp, h):
        """E[p^2] selector MM at stats rows 32-63; deferred until the
        gpsimd square has landed.  First sumsq write must start=True
        (has_written clearing is (written partitions) x (whole bank), so
        the mean rows' start does NOT cover rows 32-63); the last sel_sq
        closes the bank's coarse group so the mean copy passes the sim's
        open-group read check."""
        tp = pend["t"] % 2
        nc.tensor.matmul(
            out=grp["st"][32:64, :], lhsT=selv[:, 2 * tp + h, :],
            rhs=pend[f"sq{h}"],
            start=(tp == 0 and h == 0), stop=(tp == 1 and h == 1),
            tile_position=(0, 32), skip_group_check=True,
        )

    def stats_copy_m2(grp):
        # reads rows 0-63 (mean + open sumsq) so the tile tracker orders it
        # against the sel_sq PE writes sharing the bank; rows 32-63 unused
        mean_sb = msp.tile([64, TILE], f32, tag="mean")
        epilogue(mean_sb, grp["st"][0:64, :], 0.0, relu=False, force="act")
        grp["mean_sb"] = mean_sb
        m2t = msp.tile([32, TILE], f16, tag="m2")
        if USE_GPS_M2 and not grp.get("last"):
            nc.gpsimd.tensor_mul(m2t, mean_sb[0:32, :], mean_sb[0:32, :])
        else:
            nc.vector.tensor_mul(m2t, mean_sb[0:32, :], mean_sb[0:32, :])
            eng_ns["dve"] += (208 + TILE) / 0.96
        grp["m2t"] = m2t
        for tp in range(2):
            x0 = grp["x0"][tp]
            for h in range(2):
                r = 16 * tp + 8 * h
                nc.sync.dma_start(
                    out=meant[:, x0 + h * TILE : x0 + (h + 1) * TILE],
                    in_=mean_sb[r : r + 8, :],
                )

    def stats_negmm_sqrt(grp):
        st = grp["st"]
        std_sb = msp.tile([32, TILE], f32, tag="std")
        if DEBUG_EPSQ:
            nc.scalar.activation(
                out=std_sb, in_=st[32:64, :], func=AF.Identity, scale=1.0
            )
            eng_ns["act"] += _act_cost(TILE)
            grp["std_sb"] = std_sb
            return
        # st[32:64] -= mean^2  (PE accumulate; the coarse group was already
        # closed by the last sel_sq, so bypass the checker)
        nc.tensor.matmul(
            out=st[32:64, :], lhsT=negd, rhs=grp["m2t"],
            start=False, stop=True, tile_position=(0, 32),
            skip_group_check=True,
        )
        # std = sqrt(16/15 * (E[p^2] - mean^2))
        nc.scalar.activation(
            out=std_sb, in_=st[32:64, :], func=AF.Sqrt, scale=16.0 / 15.0,
        )
        eng_ns["act"] += _act_cost(TILE)
        grp["std_sb"] = std_sb

    def stats_dma(grp):
        for tp in range(2):
            x0 = grp["x0"][tp]
            for h in range(2):
                r = 16 * tp + 8 * h
                nc.sync.dma_start(
                    out=stdt[:, x0 + h * TILE : x0 + (h + 1) * TILE],
                    in_=grp["std_sb"][r : r + 8, :],
                )

    def xt_fetch(t, eng=None):
        x0 = t * DTILE
        tile = xp.tile([128, 2, TILE], f16, tag="xt", name=f"xt_{t}")
        (eng or nc.sync).dma_start(
            out=tile,
            in_=xt[:, x0 : x0 + DTILE].rearrange("p (h n) -> p h n", n=TILE),
        )
        return tile

    # Software pipeline with fine-grained interleaving: dtile t-1's L4,
    # preds, and stats-chain links are woven between dtile t's duo bursts.
    # Each deferred block's dependencies resolved >=2 phases earlier, so it
    # never stalls the PE FIFO head, and it gives the PE dependency-free
    # filler work during epilogue drain waits (keeps HAM warm).
    # Input tiles are prefetched 2 dtiles ahead (xp bufs=3) so L1 never
    # waits on the inbound DMA.
    # One-deep deferral, spread thin: dtile t-1's L4 halves, preds
    # epilogues, squares, and stats selectors are woven through iter t as
    # PE/engine filler (each dep resolved >= 2 phases earlier); the 1-bank
    # p4 tile pings h0/h1 through the same bank outside the php rotation,
    # so the L1-3 rotation stays a clean 8-alloc-per-phase period.  The
    # stats chain for a 2-dtile group runs early in the following
    # iteration — entirely BEFORE the next group's first sel_mean, since
    # with stp bufs=1 a later-emitted reader of the old bank would race
    # the new group's matmuls.
    xt_tiles = {0: xt_fetch(0), 1: xt_fetch(1)}
    pend = None
    cgrp = None  # group whose copy/m2/negmm/sqrt chain runs this iter
    grp = None
    for t in range(n_dt + 2):
        run = t < n_dt
        if run:
            x0 = t * DTILE
            for pf in (t + 2, t + 3):
                if pf < n_dt and pf not in xt_tiles:
                    xt_tiles[pf] = xt_fetch(pf)
            xt_t = xt_tiles.pop(t)
            h1, h2, h3 = {}, {}, {}
            layer_duo(1, t, xt_t, None, h1, 0)
        if pend is not None:
            l4_mms(pend, 0)
        if run:
            layer_duo(1, t, xt_t, None, h1, 1)
        if pend is not None:
            l4_pr(pend, 0)
        if cgrp is not None:
            stats_copy_m2(cgrp)
        if run:
            layer_duo(1, t, xt_t, None, h1, 2)
        if pend is not None:
            l4_mms(pend, 1)
            sq_gps(pend, 0)
        if run:
            layer_duo(1, t, xt_t, None, h1, 3)
        if pend is not None:
            l4_pr(pend, 1)
        if cgrp is not None:
            stats_negmm_sqrt(cgrp)
        if run:
            layer_duo(2, t, None, h1, h2, 0)
        if pend is not None:
            sq_gps(pend, 1)
            if pend["t"] % 2 == 0:
                grp = {"x0": {}}
            grp["x0"][pend["t"] % 2] = pend["x0"]
            pend["grp"] = grp
            sel_mean(pend, grp, 0)
        if run:
            layer_duo(2, t, None, h1, h2, 1)
        if cgrp is not None:
            stats_dma(cgrp)
        if run:
            layer_duo(2, t, None, h1, h2, 2)
        if pend is not None:
            sel_mean(pend, grp, 1)
        if run:
            layer_duo(2, t, None, h1, h2, 3)
            layer_duo(3, t, None, h2, h3, 0)
        if pend is not None:
            sel_sq(pend, grp, 0)
        if run:
            layer_duo(3, t, None, h2, h3, 1)
            layer_duo(3, t, None, h2, h3, 2)
        if pend is not None:
            sel_sq(pend, grp, 1)
        if run:
            layer_duo(3, t, None, h2, h3, 3)
        cgrp = None
        if pend is not None and pend["t"] % 2 == 1:
            cgrp = pend["grp"]
            if t >= n_dt:
                cgrp["last"] = True
        if run:
            pend = {"x0": x0, "t": t, "h3": h3}
            if t == n_dt - 1:
                # tail: the final dtile's squares run on the then-idle
                # Vector engine instead of the slower gpsimd
                pend["last"] = True
        else:
            pend = None


def build(b_core=B_CORE, num_devices=N_CORES):
    from contextlib import ExitStack

    import concourse.bacc as bacc
    import concourse.tile as tile
    from concourse import mybir

    f32 = mybir.dt.float32
    f16 = mybir.dt.float16
    nc = bacc.Bacc(
        "TRN2", target_bir_lowering=False, debug=False, num_devices=num_devices
    )
    xt = nc.dram_tensor("xt", [128, b_core], f16, kind="ExternalInput").ap()
    wr = nc.dram_tensor("wpackr", [128, WR], f16, kind="ExternalInput").ap()
    wb = nc.dram_tensor("wpackb", [128, WB], f32, kind="ExternalInput").ap()
    meant = nc.dram_tensor("meant", [8, b_core], f32, kind="ExternalOutput").ap()
    stdt = nc.dram_tensor("stdt", [8, b_core], f32, kind="ExternalOutput").ap()
    with tile.TileContext(nc) as tc:
        with ExitStack() as ctx:
            _emit(tc, ctx, xt, wr, wb, meant, stdt, b_core)
    nc.compile()
    return nc


_NC_CACHE = {}


def kernel(x, W1, b1, W2, b2, W3, b3, W4, b4):
    from concourse.bass_utils import run_bass_kernel_spmd

    key = ("full", B_CORE)
    if key not in _NC_CACHE:
        _NC_CACHE[key] = build(B_CORE, N_CORES)
    nc = _NC_CACHE[key]

    xt_per_core, wpackr, wpackb = pack_inputs(
        np.asarray(x), np.asarray(W1), np.asarray(b1), np.asarray(W2),
        np.asarray(b2), np.asarray(W3), np.asarray(b3), np.asarray(W4),
        np.asarray(b4),
    )
    in_maps = [
        {"xt": xt_per_core[c], "wpackr": wpackr, "wpackb": wpackb}
        for c in range(N_CORES)
    ]
    # transient device wedges (NRT_EXEC_UNIT_UNRECOVERABLE etc.) recover on
    # retry, per the TRN2 runtime docs; don't let one sink the whole call
    import os
    res = None
    for attempt in range(3):
        try:
            res = run_bass_kernel_spmd(nc, in_maps, list(range(N_CORES))).results
            break
        except Exception:
            if attempt == 2:
                raise
            os.environ.setdefault("NEURON_RT_RESET_CORES", "1")
    mean = np.concatenate([res[c]["meant"] for c in range(N_CORES)], axis=1).T
    std = np.concatenate([res[c]["stdt"] for c in range(N_CORES)], axis=1).T
    return np.ascontiguousarray(mean), np.ascontiguousarray(std)



# revision 38
# speedup vs baseline: 1.0089x; 1.0018x over previous
"""Ensemble MLP surrogate (16 models, 32->64->64->64->8, relu) on 8 TRN2 cores.

Data-parallel over batch, weights replicated.  Feature-on-partition layout,
batch streamed as the matmul moving operand in fp16.  Per double-tile of 1024
batch elements the schedule is epilogue-bound (PSUM->SBUF bias+ReLU on the
Scalar/Vector engines), so the design centers on keeping those two engines
saturated:

  - L1 runs as 8-MM octets (two model-pairs concurrently, all 16 PE array
    tiles busy) via parity-swapped W1 packing for odd pairs; L2/L3 as 4-MM
    quads; L4 packs all 16 models' preds into the full 128 partitions
    (pairs j and j+4 share a col tile via PSUM accumulation), halves
    pinging through a single dedicated PSUM bank OUTSIDE the main
    rotation, so the L1-3 pipeline keeps a clean 8-alloc-per-phase period
    (this alone was worth ~35us: a long-lived L4 tile inside the rotation
    stalls L1 every dtile and HAM-cold-clocks the PE).
  - PSUM (8 banks): php bufs=3 x [128,2,512] (6) + p4 1 x [128,512] (1) +
    stats 1 x [64,512] (1).  Mean/sumsq for TWO dtiles accumulate into the
    stats bank at disjoint 8-row bands via column-shifted selector
    matmuls, so the mean-copy/mean^2/sqrt chain runs once per two dtiles
    on [64,512] instead of per-dtile on [8,1024].  With stp bufs=1 the
    whole chain must be EMITTED before the next group's first sel_mean
    (later-emitted readers of a recycled bank race the new group's MMs).
  - The first write to each PSUM partition-region needs start=True:
    has_written clearing is (written partitions) x (whole bank), NOT
    bank-global — a region first written with start=False accumulates
    onto stale PSUM garbage.  skip_group_check where the sim's
    partition-coarse zero-region tracker would false-flag.
  - Squares (pred^2, mean^2) run on the otherwise-idle GPSIMD engine; the
    E[p^2]-mean^2 subtraction is a -identity accumulating matmul on the PE.
  - Each duo's two bias+ReLU epilogues go to DIFFERENT engines (Scalar /
    Vector) so the php slot frees after the parallel max, not a serial sum.
  - Input and output DMA on sync (HWDGE).
"""

import numpy as np

N_MODELS = 16
IN_DIM = 32
HID = 64
OUT_DIM = 8
BATCH = 131072
N_CORES = 8
B_CORE = BATCH // N_CORES  # 16384
TILE = 512  # PSUM bank limit on matmul out cols (fp32)
DTILE = 2 * TILE  # batch elements per pipeline step
NPAIR = N_MODELS // 2

# wpackr free-dim layout (fp16 matmul operands, 128 partitions)
OFF_W1 = 0  # [128, 8, 64]  odd pairs parity-swapped (rows b,a,b,a)
OFF_W2 = OFF_W1 + NPAIR * 64  # [128, 8, 64] canonical (a; b)
OFF_W3 = OFF_W2 + NPAIR * 64  # [128, 8, 64] odd pairs input-swapped (b; a)
OFF_W4 = OFF_W3 + NPAIR * 64  # [128, 8, 32] pair cols 16(j//4)+8c+o
OFF_SELV = OFF_W4 + NPAIR * 32  # [128, 4, 32] band-v selector (1/16)
OFF_NEGD = OFF_SELV + 4 * 32  # [128, 32] -I32 (rows 0-31)
WR = OFF_NEGD + 32
# wpackb free-dim layout (fp32 biases)
OFF_B1 = 0  # [128, 8]
OFF_B2 = OFF_B1 + NPAIR  # [128, 8] (parity-swapped odd pairs)
OFF_B3 = OFF_B2 + NPAIR  # [128, 8]
OFF_B4 = OFF_B3 + NPAIR  # [128, 1] all-model packed preds bias
WB = OFF_B4 + 1

USE_GPS_SQ = True  # pred squares on GPSIMD (else Vector)
USE_GPS_M2 = True  # mean^2 on GPSIMD (else Vector)
DEBUG_EPSQ = False  # skip negmm; stdt outputs raw E[p^2] for debugging


# epilogue op costs (ns) for greedy ACT/DVE load balancing (HW-measured).
# Note: the resulting Scalar-over-Vector busy skew (~9us/kernel) is optimal;
# equalizing busy by biasing ACT's cost shifts ops to the slower-per-op DVE
# (+~170ns each of total work) and measures worse.
def _act_cost(fd):
    return (315 + fd) / 1.2


def _dve_cost(fd):
    return (208 + fd) / 0.96


def pack_inputs(x, W1, b1, W2, b2, W3, b3, W4, b4, b_core=B_CORE, n_cores=N_CORES):
    """Host-side packing. Returns (xt_per_core list, wpackr fp16, wpackb f32)."""
    f32 = np.float32
    x = np.ascontiguousarray(x, dtype=f32)
    wpack = np.zeros((128, WR), f32)
    wpackb = np.zeros((128, WB), f32)

    w1v = wpack[:, OFF_W1 : OFF_W1 + NPAIR * 64].reshape(128, NPAIR, 64)
    w2v = wpack[:, OFF_W2 : OFF_W2 + NPAIR * 64].reshape(128, NPAIR, 64)
    w3v = wpack[:, OFF_W3 : OFF_W3 + NPAIR * 64].reshape(128, NPAIR, 64)
    w4v = wpack[:, OFF_W4 : OFF_W4 + NPAIR * 32].reshape(128, NPAIR, 32)
    for j in range(NPAIR):
        a, b = 2 * j, 2 * j + 1
        for k in range(4):
            # L1 row-group k: even pairs rows a,b,a,b; odd pairs b,a,b,a so the
            # swapped-col octet positions still land both outputs canonically
            if j % 2 == 0:
                w1v[32 * k : 32 * k + 32, j, :] = W1[a if k % 2 == 0 else b]
            else:
                w1v[32 * k : 32 * k + 32, j, :] = W1[b if k % 2 == 0 else a]
        w2v[0:HID, j, :] = W2[a]
        w2v[HID:128, j, :] = W2[b]
        if j % 2 == 0:  # h2 canonical input
            w3v[0:HID, j, :] = W3[a]
            w3v[HID:128, j, :] = W3[b]
        else:  # h2 swapped input (slot0 = model b)
            w3v[0:HID, j, :] = W3[b]
            w3v[HID:128, j, :] = W3[a]
        # pair j's preds land at partitions 32*(j%4) + 16*(j//4) + 8c + o:
        # all 16 models x 8 outs tile the full 128 partitions of one p4 tile
        w4v[0:HID, j, 16 * (j // 4) : 16 * (j // 4) + OUT_DIM] = W4[a]
        w4v[HID:128, j, 16 * (j // 4) + OUT_DIM : 16 * (j // 4) + 16] = W4[b]

    selv = wpack[:, OFF_SELV : OFF_SELV + 4 * 32].reshape(128, 4, 32)
    negd = wpack[:, OFF_NEGD : OFF_NEGD + 32]
    b4v = wpackb[:, OFF_B4]
    for j in range(NPAIR):
        for c in range(2):  # model-within-pair
            for o in range(OUT_DIM):
                p = 32 * (j % 4) + 16 * (j // 4) + 8 * c + o
                b4v[p] = b4[2 * j + c, o]
                for v in range(4):  # stats band (t'=v//2, h=v%2)
                    selv[p, v, 8 * v + o] = 1.0 / 16.0  # exact in fp16
    for r in range(32):
        negd[r, r] = -1.0
    for j in range(NPAIR):
        a, b = 2 * j, 2 * j + 1
        wpackb[0:HID, OFF_B1 + j] = b1[a]
        wpackb[HID:128, OFF_B1 + j] = b1[b]
        # h2 output of odd pairs is parity-swapped (slot0 = model b)
        wpackb[0:HID, OFF_B2 + j] = b2[a if j % 2 == 0 else b]
        wpackb[HID:128, OFF_B2 + j] = b2[b if j % 2 == 0 else a]
        wpackb[0:HID, OFF_B3 + j] = b3[a]
        wpackb[HID:128, OFF_B3 + j] = b3[b]

    wpack16 = wpack.astype(np.float16)
    x16 = x.astype(np.float16)
    xt_per_core = []
    for c in range(n_cores):
        shard = x16[c * b_core : (c + 1) * b_core]  # [b_core, 32]
        xt = np.ascontiguousarray(np.tile(shard.T, (4, 1)))  # [128, b_core]
        xt_per_core.append(xt)
    return xt_per_core, wpack16, wpackb


def _emit(tc, ctx, xt, wr, wb, meant, stdt, b_core):
    import concourse.bass as bass  # noqa: F401
    from concourse import mybir

    nc = tc.nc
    f32 = mybir.dt.float32
    f16 = mybir.dt.float16
    AF = mybir.ActivationFunctionType
    ALU = mybir.AluOpType

    n_dt = b_core // DTILE

    consts = ctx.enter_context(tc.tile_pool(name="consts", bufs=1))
    xp = ctx.enter_context(tc.tile_pool(name="xp", bufs=4))
    hp = [
        ctx.enter_context(tc.tile_pool(name=f"h{i}p", bufs=10)) for i in range(3)
    ]
    prp = ctx.enter_context(tc.tile_pool(name="prp", bufs=6))
    sqp = ctx.enter_context(tc.tile_pool(name="sqp", bufs=6))
    msp = ctx.enter_context(tc.tile_pool(name="msp", bufs=4))  # mean/std/m2 sbuf
    # PSUM budget (8 banks): php 3x[128,2,512]=6 (hosts L1-3 pair tiles AND
    # the packed L4 preds tile, 25 allocs/dtile) + statsp 2x[64,512]=2
    # (2-dtile mean/sumsq accumulators, double-buffered).
    # (A 4-deep php was tried: the extra slack lets the PE sprint then idle
    # past the HAM window -> 52% cold-clock; 3-deep backpressure paces it.)
    php = ctx.enter_context(tc.tile_pool(name="php", bufs=3, space="PSUM"))
    p4p = ctx.enter_context(tc.tile_pool(name="p4p", bufs=1, space="PSUM"))
    stp = ctx.enter_context(tc.tile_pool(name="stp", bufs=1, space="PSUM"))

    # Split the weight-pack load so the first L1 matmul is gated only by
    # the L1 weights + biases + xt0 (~0.5MB of later-needed weights would
    # otherwise transfer ahead of xt0 on the serial sync DMA queue).  The
    # remainder (w2v onward, first needed mid-iter-0) follows xt0/xt1.
    cw = consts.tile([128, WR], f16)
    nc.sync.dma_start(out=cw[:, OFF_W1 : OFF_W2], in_=wr[:, OFF_W1 : OFF_W2])
    cwb = consts.tile([128, WB], f32)
    nc.sync.dma_start(out=cwb, in_=wb)
    # tiny warmup activations: trigger both ACT table-set loads at kernel
    # start (overlapped with the const/input DMAs) instead of stalling the
    # first real epilogue ~2.6us
    warm = consts.tile([128, 2], f32, tag="warm")
    nc.scalar.activation(warm[:, 0:1], cwb[:, 0:1], AF.Relu)
    # sqrt warmup reads the relu output so its input is >= 0 (sim asserts)
    nc.scalar.activation(warm[:, 1:2], warm[:, 0:1], AF.Sqrt)
    w1v = cw[:, OFF_W1 : OFF_W1 + NPAIR * 64].rearrange("p (j f) -> p j f", f=64)
    w2v = cw[:, OFF_W2 : OFF_W2 + NPAIR * 64].rearrange("p (j f) -> p j f", f=64)
    w3v = cw[:, OFF_W3 : OFF_W3 + NPAIR * 64].rearrange("p (j f) -> p j f", f=64)
    w4v = cw[:, OFF_W4 : OFF_W4 + NPAIR * 32].rearrange("p (j f) -> p j f", f=32)
    selv = cw[:, OFF_SELV : OFF_SELV + 4 * 32].rearrange("p (v f) -> p v f", f=32)
    negd = cw[0:32, OFF_NEGD : OFF_NEGD + 32]

    # greedy engine balancer for PSUM->SBUF epilogues
    eng_ns = {"act": 0.0, "dve": 0.0}

    def epilogue(out, in_, bias, relu, force=None):
        fd = out.free_size()
        use_act = (
            force == "act"
            if force
            else eng_ns["act"] + _act_cost(fd) <= eng_ns["dve"] + _dve_cost(fd)
        )
        if use_act:
            eng_ns["act"] += _act_cost(fd)
            nc.scalar.activation(
                out, in_, AF.Relu if relu else AF.Identity, bias=bias, scale=1.0
            )
        else:
            eng_ns["dve"] += _dve_cost(fd)
            if relu:
                nc.vector.tensor_scalar(
                    out, in_, bias, 0.0, op0=ALU.add, op1=ALU.max
                )
            else:
                nc.vector.tensor_scalar(out, in_, bias, None, op0=ALU.add)

    def l1_octet(d, xt_t, ph):
        j0, j1 = 2 * d, 2 * d + 1
        for k in range(4):
            rg = 32 * k
            c0 = 64 * (k % 2)  # even pair col half
            c1 = 64 * ((k + 1) % 2)  # odd pair col half (swapped)
            nc.tensor.matmul(
                out=ph[j0][c0 : c0 + 64, k // 2, :],
                lhsT=w1v[rg : rg + 32, j0, :],
                rhs=xt_t[rg : rg + 32, k // 2, :],
                start=True, stop=True, tile_position=(rg, c0),
            )
            nc.tensor.matmul(
                out=ph[j1][c1 : c1 + 64, k // 2, :],
                lhsT=w1v[rg : rg + 32, j1, :],
                rhs=xt_t[rg : rg + 32, k // 2, :],
                start=True, stop=True, tile_position=(rg, c1),
            )

    def mid_quads(d, wv, hin, ph):
        # N is hard-capped at 512 per matmul: the ISA rejects a 2-bank
        # N=1024 out AP (codegen assert 's3d3_mm_num_elements') — verified
        j0, j1 = 2 * d, 2 * d + 1
        for h in range(2):
            nc.tensor.matmul(
                out=ph[j0][0:64, h, :], lhsT=wv[0:64, j0, :],
                rhs=hin[j0][0:64, h, :], start=True, stop=True,
                tile_position=(0, 0),
            )
            nc.tensor.matmul(
                out=ph[j0][64:128, h, :], lhsT=wv[64:128, j0, :],
                rhs=hin[j0][64:128, h, :], start=True, stop=True,
                tile_position=(64, 64),
            )
            nc.tensor.matmul(
                out=ph[j1][64:128, h, :], lhsT=wv[0:64, j1, :],
                rhs=hin[j1][0:64, h, :], start=True, stop=True,
                tile_position=(0, 64),
            )
            nc.tensor.matmul(
                out=ph[j1][0:64, h, :], lhsT=wv[64:128, j1, :],
                rhs=hin[j1][64:128, h, :], start=True, stop=True,
                tile_position=(64, 0),
            )

    def layer_duo(lnum, t, xt_t, hin, hout, d):
        """One duo (2 pairs) of one layer: an 8-MM PE burst + 2 epilogues."""
        wv = (None, w2v, w3v)[lnum - 1]
        boff = (OFF_B1, OFF_B2, OFF_B3)[lnum - 1]
        j0, j1 = 2 * d, 2 * d + 1
        ph = {}
        for j in (j0, j1):
            ph[j] = php.tile(
                [128, 2, TILE], f32, tag="ph", name=f"ph{lnum}_{t}_{j}"
            )
        if lnum == 1:
            l1_octet(d, xt_t, ph)
        else:
            mid_quads(d, wv, hin, ph)
        for idx, j in enumerate((j0, j1)):
            hout[j] = hp[lnum - 1].tile(
                [128, 2, TILE], f16, tag=f"h{lnum}", name=f"h{lnum}_{t}_{j}"
            )
            # the duo's two drains go to DIFFERENT engines so they run in
            # parallel: the php slot frees after max(act, dve) ~1.27us
            # instead of a possible 2.5us serial drain on one engine
            epilogue(
                hout[j].rearrange("p h n -> p (h n)"),
                ph[j].rearrange("p h n -> p (h n)"),
                cwb[:, boff + j : boff + j + 1],
                relu=True,
                force="act" if idx == 0 else "dve",
            )

    def l4_mms(pend, h):
        """Packed L4 for half h of the previous dtile: all 16 models' preds
        tile the full 128 partitions of a 1-bank p4 tile (halves ping
        through the same bank).  Pair j lands at partitions
        32*(j%4)+16*(j//4)+8c+o; pairs j and j+4 share a col tile via PSUM
        accumulation (their lhsT cols are disjoint, rest zeros)."""
        h3 = pend["h3"]
        p4h = p4p.tile([128, TILE], f32, tag="p4", name=f"p4_{pend['t']}_{h}")
        pend[f"p4{h}"] = p4h
        # skip_group_check: the sim's zero-region tracker drops the
        # partition base, so the four partition-disjoint col groups of this
        # bank alias to one flag; HW has_written is per-partition and fine.
        for j in range(NPAIR):
            q = j % 4
            nc.tensor.matmul(
                out=p4h[32 * q : 32 * q + 32, :],
                lhsT=w4v[:, j, :],
                rhs=h3[j][:, h, :],
                start=(j < 4), stop=(j >= 4),
                tile_position=(0, 32 * q),
                skip_group_check=True,
            )

    def l4_pr(pend, h):
        prh = prp.tile([128, TILE], f16, tag="pr", name=f"pr_{pend['t']}_{h}")
        epilogue(
            prh, pend[f"p4{h}"], cwb[:, OFF_B4 : OFF_B4 + 1],
            relu=False, force="act",
        )
        pend[f"pr{h}"] = prh

    def sq_gps(pend, h):
        prh = pend[f"pr{h}"]
        sqh = sqp.tile([128, TILE], f16, tag="sq", name=f"sq_{pend['t']}_{h}")
        if USE_GPS_SQ and not pend.get("last"):
            nc.gpsimd.tensor_mul(sqh, prh, prh)
        else:
            nc.vector.tensor_mul(sqh, prh, prh)
            eng_ns["dve"] += (208 + TILE) / 0.96
        pend[f"sq{h}"] = sqh

    def sel_mean(pend, grp, h):
        """Mean selector MM into the 2-dtile stats bank.  Band v=2t'+h of
        selv places dtile t', half h at partitions 8v..8v+8 (zeros elsewhere
        in the 32-row block accumulate harmlessly)."""
        tp = pend["t"] % 2
        if tp == 0 and h == 0:
            grp["st"] = stp.tile(
                [64, TILE], f32, tag="st", name=f"st_{pend['t'] // 2}"
            )
        nc.tensor.matmul(
            out=grp["st"][0:32, :], lhsT=selv[:, 2 * tp + h, :],
            rhs=pend[f"pr{h}"],
            start=(tp == 0 and h == 0), stop=False, tile_position=(0, 0),
            skip_group_check=True,
        )

    def sel_sq(pend, grp, h):
        """E[p^2] selector MM at stats rows 32-63; deferred until the
        gpsimd square has landed.  First sumsq write must start=True
        (has_written clearing is (written partitions) x (whole bank), so
        the mean rows' start does NOT cover rows 32-63); the last sel_sq
        closes the bank's coarse group so the mean copy passes the sim's
        open-group read check."""
        tp = pend["t"] % 2
        nc.tensor.matmul(
            out=grp["st"][32:64, :], lhsT=selv[:, 2 * tp + h, :],
            rhs=pend[f"sq{h}"],
            start=(tp == 0 and h == 0), stop=(tp == 1 and h == 1),
            tile_position=(0, 32), skip_group_check=True,
        )

    def stats_copy_m2(grp):
        # reads rows 0-63 (mean + open sumsq) so the tile tracker orders it
        # against the sel_sq PE writes sharing the bank; rows 32-63 unused
        mean_sb = msp.tile([64, TILE], f32, tag="mean")
        epilogue(mean_sb, grp["st"][0:64, :], 0.0, relu=False, force="act")
        grp["mean_sb"] = mean_sb
        m2t = msp.tile([32, TILE], f16, tag="m2")
        if USE_GPS_M2 and not grp.get("last"):
            nc.gpsimd.tensor_mul(m2t, mean_sb[0:32, :], mean_sb[0:32, :])
        else:
            nc.vector.tensor_mul(m2t, mean_sb[0:32, :], mean_sb[0:32, :])
            eng_ns["dve"] += (208 + TILE) / 0.96
        grp["m2t"] = m2t
        for tp in range(2):
            x0 = grp["x0"][tp]
            for h in range(2):
                r = 16 * tp + 8 * h
                nc.sync.dma_start(
                    out=meant[:, x0 + h * TILE : x0 + (h + 1) * TILE],
                    in_=mean_sb[r : r + 8, :],
                )

    def stats_negmm_sqrt(grp):
        st = grp["st"]
        std_sb = msp.tile([32, TILE], f32, tag="std")
        if DEBUG_EPSQ:
            nc.scalar.activation(
                out=std_sb, in_=st[32:64, :], func=AF.Identity, scale=1.0
            )
            eng_ns["act"] += _act_cost(TILE)
            grp["std_sb"] = std_sb
            return
        # st[32:64] -= mean^2  (PE accumulate; the coarse group was already
        # closed by the last sel_sq, so bypass the checker)
        nc.tensor.matmul(
            out=st[32:64, :], lhsT=negd, rhs=grp["m2t"],
            start=False, stop=True, tile_position=(0, 32),
            skip_group_check=True,
        )
        # std = sqrt(16/15 * (E[p^2] - mean^2))
        nc.scalar.activation(
            out=std_sb, in_=st[32:64, :], func=AF.Sqrt, scale=16.0 / 15.0,
        )
        eng_ns["act"] += _act_cost(TILE)
        grp["std_sb"] = std_sb

    def stats_dma(grp):
        for tp in range(2):
            x0 = grp["x0"][tp]
            for h in range(2):
                r = 16 * tp + 8 * h
                nc.sync.dma_start(
                    out=stdt[:, x0 + h * TILE : x0 + (h + 1) * TILE],
                    in_=grp["std_sb"][r : r + 8, :],
                )

    def xt_fetch(t, eng=None):
        x0 = t * DTILE
        tile = xp.tile([128, 2, TILE], f16, tag="xt", name=f"xt_{t}")
        (eng or nc.sync).dma_start(
            out=tile,
            in_=xt[:, x0 : x0 + DTILE].rearrange("p (h n) -> p h n", n=TILE),
        )
        return tile

    # Software pipeline with fine-grained interleaving: dtile t-1's L4,
    # preds, and stats-chain links are woven between dtile t's duo bursts.
    # Each deferred block's dependencies resolved >=2 phases earlier, so it
    # never stalls the PE FIFO head, and it gives the PE dependency-free
    # filler work during epilogue drain waits (keeps HAM warm).
    # Input tiles are prefetched 2 dtiles ahead (xp bufs=3) so L1 never
    # waits on the inbound DMA.
    # One-deep deferral, spread thin: dtile t-1's L4 halves, preds
    # epilogues, squares, and stats selectors are woven through iter t as
    # PE/engine filler (each dep resolved >= 2 phases earlier); the 1-bank
    # p4 tile pings h0/h1 through the same bank outside the php rotation,
    # so the L1-3 rotation stays a clean 8-alloc-per-phase period.  The
    # stats chain for a 2-dtile group runs early in the following
    # iteration — entirely BEFORE the next group's first sel_mean, since
    # with stp bufs=1 a later-emitted reader of the old bank would race
    # the new group's matmuls.
    xt_tiles = {0: xt_fetch(0), 1: xt_fetch(1)}
    # rest of the weight pack (w2v/w3v/w4v/selv/negd), behind xt0/xt1
    nc.sync.dma_start(out=cw[:, OFF_W2 : WR], in_=wr[:, OFF_W2 : WR])
    pend = None
    cgrp = None  # group whose copy/m2/negmm/sqrt chain runs this iter
    grp = None
    for t in range(n_dt + 2):
        run = t < n_dt
        if run:
            x0 = t * DTILE
            for pf in (t + 2, t + 3):
                if pf < n_dt and pf not in xt_tiles:
                    xt_tiles[pf] = xt_fetch(pf)
            xt_t = xt_tiles.pop(t)
            h1, h2, h3 = {}, {}, {}
            layer_duo(1, t, xt_t, None, h1, 0)
        if pend is not None:
            l4_mms(pend, 0)
        if run:
            layer_duo(1, t, xt_t, None, h1, 1)
        if pend is not None:
            l4_pr(pend, 0)
        if cgrp is not None:
            stats_copy_m2(cgrp)
        if run:
            layer_duo(1, t, xt_t, None, h1, 2)
        if pend is not None:
            l4_mms(pend, 1)
            sq_gps(pend, 0)
        if run:
            layer_duo(1, t, xt_t, None, h1, 3)
        if pend is not None:
            l4_pr(pend, 1)
        if cgrp is not None:
            stats_negmm_sqrt(cgrp)
        if run:
            layer_duo(2, t, None, h1, h2, 0)
        if pend is not None:
            sq_gps(pend, 1)
            if pend["t"] % 2 == 0:
                grp = {"x0": {}}
            grp["x0"][pend["t"] % 2] = pend["x0"]
            pend["grp"] = grp
            sel_mean(pend, grp, 0)
        if run:
            layer_duo(2, t, None, h1, h2, 1)
        if cgrp is not None:
            stats_dma(cgrp)
        if run:
            layer_duo(2, t, None, h1, h2, 2)
        if pend is not None:
            sel_mean(pend, grp, 1)
        if run:
            layer_duo(2, t, None, h1, h2, 3)
            layer_duo(3, t, None, h2, h3, 0)
        if pend is not None:
            sel_sq(pend, grp, 0)
        if run:
            layer_duo(3, t, None, h2, h3, 1)
            layer_duo(3, t, None, h2, h3, 2)
        if pend is not None:
            sel_sq(pend, grp, 1)
        if run:
            layer_duo(3, t, None, h2, h3, 3)
        cgrp = None
        if pend is not None and pend["t"] % 2 == 1:
            cgrp = pend["grp"]
            if t >= n_dt:
                cgrp["last"] = True
        if run:
            pend = {"x0": x0, "t": t, "h3": h3}
            if t == n_dt - 1:
                # tail: the final dtile's squares run on the then-idle
                # Vector engine instead of the slower gpsimd
                pend["last"] = True
        else:
            pend = None


def build(b_core=B_CORE, num_devices=N_CORES):
    from contextlib import ExitStack

    import concourse.bacc as bacc
    import concourse.tile as tile
    from concourse import mybir

    f32 = mybir.dt.float32
    f16 = mybir.dt.float16
    nc = bacc.Bacc(
        "TRN2", target_bir_lowering=False, debug=False, num_devices=num_devices
    )
    xt = nc.dram_tensor("xt", [128, b_core], f16, kind="ExternalInput").ap()
    wr = nc.dram_tensor("wpackr", [128, WR], f16, kind="ExternalInput").ap()
    wb = nc.dram_tensor("wpackb", [128, WB], f32, kind="ExternalInput").ap()
    meant = nc.dram_tensor("meant", [8, b_core], f32, kind="ExternalOutput").ap()
    stdt = nc.dram_tensor("stdt", [8, b_core], f32, kind="ExternalOutput").ap()
    with tile.TileContext(nc) as tc:
        with ExitStack() as ctx:
            _emit(tc, ctx, xt, wr, wb, meant, stdt, b_core)
    nc.compile()
    return nc


_NC_CACHE = {}


def kernel(x, W1, b1, W2, b2, W3, b3, W4, b4):
    from concourse.bass_utils import run_bass_kernel_spmd

    key = ("full", B_CORE)
    if key not in _NC_CACHE:
        _NC_CACHE[key] = build(B_CORE, N_CORES)
    nc = _NC_CACHE[key]

    xt_per_core, wpackr, wpackb = pack_inputs(
        np.asarray(x), np.asarray(W1), np.asarray(b1), np.asarray(W2),
        np.asarray(b2), np.asarray(W3), np.asarray(b3), np.asarray(W4),
        np.asarray(b4),
    )
    in_maps = [
        {"xt": xt_per_core[c], "wpackr": wpackr, "wpackb": wpackb}
        for c in range(N_CORES)
    ]
    # transient device wedges (NRT_EXEC_UNIT_UNRECOVERABLE etc.) recover on
    # retry, per the TRN2 runtime docs; don't let one sink the whole call
    import os
    res = None
    for attempt in range(3):
        try:
            res = run_bass_kernel_spmd(nc, in_maps, list(range(N_CORES))).results
            break
        except Exception:
            if attempt == 2:
                raise
            os.environ.setdefault("NEURON_RT_RESET_CORES", "1")
    mean = np.concatenate([res[c]["meant"] for c in range(N_CORES)], axis=1).T
    std = np.concatenate([res[c]["stdt"] for c in range(N_CORES)], axis=1).T
    return np.ascontiguousarray(mean), np.ascontiguousarray(std)

